# revision 24
# baseline (speedup 1.0000x reference)
"""Trainium kernel for nn_GATheadClassifier: cdist -> Prim MST -> 3x SSGConv -> pool -> MLP.

Self-contained: builds a Bass program (8-core SPMD, 2 graphs per core),
weights baked into the NEFF as constants, features streamed as fp8_e4m3.
First call compiles + runs via run_bass_kernel_spmd; subsequent calls
reuse a persistent jitted executable (same NEFF pipeline, no recompile).
"""
import hashlib
import numpy as np
import ml_dtypes

import jax
import concourse.bass as bass
import concourse.mybir as mybir
import concourse.tile as tile_mod
from concourse.bass import ds
from concourse.bass_utils import run_bass_kernel_spmd
from concourse.bass2jax import (
    _bass_exec_p,
    partition_id_tensor,
    install_neuronx_cc_hook,
)
from concourse.tile import TileContext
from concourse.masks import make_identity
from jax.sharding import Mesh, PartitionSpec
from jax.experimental.shard_map import shard_map

F32 = mybir.dt.float32
F16 = mybir.dt.float16
F8 = mybir.dt.float8e4
FEAT_FP8 = True  # stream features as fp8_e4m3 (4.2MB) instead of fp16 (8.4MB)
FEAT_DT = F8 if FEAT_FP8 else F16
FEAT_NP = ml_dtypes.float8_e4m3fn if FEAT_FP8 else np.float16
BF16 = mybir.dt.bfloat16
I32 = mybir.dt.int32
U32 = mybir.dt.uint32
DVE = mybir.EngineType.DVE
AX = mybir.AxisListType
AOP = mybir.AluOpType
ACTF = mybir.ActivationFunctionType

NEG = -1e30
ALPHA = 0.3
B, N, H, L = 16, 1024, 256, 8
H2 = 2 * H
NCORES = 8
GPC = B // NCORES  # graphs per core = 2
N_PRIM = N - 1     # 1023
UNROLL = 11        # 1023 = 11*93
_SECTIONS = "full"  # debug knob: "cdist" | "prim" | "full"

_MAX_WAITS = 1
_nop_n = [0]


def _patched_drain_and_barrier(self, tick_clock, wait_clock):
    nc = self.nc
    drain_inst = nc.sync.drain()
    wait_clock.add_sem_waits(
        drain_inst.ins, tile_mod.ScopedClock({None: tick_clock.global_clock})
    )
    nc.all_engine_barrier()
    assert self.sems is not None
    popped = nc._tile_sem_poison_stack.pop()
    assert popped is self._sem_poison
    nc.clear_and_free_semaphores(list(self.sems.allocated().values()))
    nc.all_engine_barrier()


tile_mod.TileContext._drain_and_barrier = _patched_drain_and_barrier


def _fix_sync_waits(nc):
    """This walrus build rejects instructions with >1 sync waits; split extras
    onto same-engine NoOps placed immediately before."""
    for func in nc.m.functions:
        for block in func.blocks:
            out = []
            changed = False
            for inst in block.instructions:
                si = inst.sync_info
                waits = list(si.on_wait) if si is not None else []
                if len(waits) > _MAX_WAITS:
                    changed = True
                    extra, keep = waits[:-_MAX_WAITS], waits[-_MAX_WAITS:]
                    for w in extra:
                        _nop_n[0] += 1
                        nop = mybir.InstNoOp(
                            name=f"waitsplit_{_nop_n[0]}", ins=[], outs=[]
                        )
                        nop.engine = inst.engine
                        nop.sync_info = mybir.SyncInfo(on_wait=[w], on_update=[])
                        try:
                            nc.register_instruction(nop)
                        except Exception:
                            pass
                        out.append(nop)
                    inst.sync_info = mybir.SyncInfo(
                        on_wait=keep, on_update=list(si.on_update)
                    )
                out.append(inst)
            if changed:
                block.instructions[:] = out


def _build(weights, n_prim=N_PRIM):
    """weights: dict of numpy arrays (W1,b1,...,Wo,bo) baked in as constants."""
    nc = bass.Bass(target_bir_lowering=False)

    feats = nc.dram_tensor("feats", [GPC, N, H], FEAT_DT, kind="ExternalInput")
    outd = nc.dram_tensor("out", [GPC, L], F32, kind="ExternalOutput")

    bf = lambda a: np.asarray(a, np.float32).astype(ml_dtypes.bfloat16)
    f32 = lambda a: np.asarray(a, np.float32)
    # weight matrices in [p, k, f] layout (contraction blocks of 128)
    def pkf(a, dt):
        a = np.asarray(a, np.float32)
        fin, fout = a.shape
        return a.reshape(fin // 128, 128, fout).transpose(1, 0, 2).astype(dt)

    W1d = nc.inline_tensor(pkf(weights["W1"], ml_dtypes.bfloat16), "W1c")
    W2d = nc.inline_tensor(pkf(weights["W2"], ml_dtypes.bfloat16), "W2c")
    W3d = nc.inline_tensor(pkf(weights["W3"], ml_dtypes.bfloat16), "W3c")
    Wdd = nc.inline_tensor(pkf(weights["Wd"], np.float32), "Wdc")
    Wod = nc.inline_tensor(pkf(weights["Wo"], np.float32), "Woc")
    brep_np = np.stack([
        np.tile(f32(weights["b1"])[None, :], (128, 1)),
        np.tile(f32(weights["b2"])[None, :], (128, 1)),
        np.tile(f32(weights["b3"])[None, :], (128, 1)),
    ], axis=1)  # [128, 3, H2]
    brepd = nc.inline_tensor(brep_np.astype(np.float32), "brepc")
    bdd = nc.inline_tensor(f32(weights["bd"])[None, :], "bdc")
    bod = nc.inline_tensor(f32(weights["bo"])[None, :], "boc")

    # DRAM scratch for row bounces
    rowscr = [nc.dram_tensor(f"rowscr{g}", [8 * N], F32) for g in range(GPC)]

    with TileContext(nc) as tc:
        with (
            tc.tile_pool(name="consts", bufs=1) as cst,
            tc.tile_pool(name="weights", bufs=1) as wts,
            tc.tile_pool(name="state", bufs=1) as st,
        ):
            ident = cst.tile([128, 128], F32)
            onesRow = cst.tile([1, 128], F32)
            onesRowP = cst.tile([33, 128], F32)
            onesCol = cst.tile([128, 1], F32)
            iotaNI = cst.tile([128, 8], I32)
            iotaN = cst.tile([128, 8], F32)
            iotaRI = cst.tile([128, N], I32)
            iotaR = cst.tile([128, N], F32)
            make_identity(nc, ident)
            nc.vector.memset(onesRow, 1.0)
            nc.vector.memset(onesRowP, 1.0)
            nc.vector.memset(onesCol, 1.0)
            nc.gpsimd.iota(iotaNI, pattern=[[128, 8]], base=0, channel_multiplier=1)
            nc.vector.tensor_copy(iotaN, iotaNI)
            nc.gpsimd.iota(iotaRI, pattern=[[1, N]], base=0, channel_multiplier=0)
            nc.vector.tensor_copy(iotaR, iotaRI)

            # weights to SBUF (bf16 for the 3 SSG layers, f32 head)
            W1 = wts.tile([128, 2, H2], BF16)
            W2 = wts.tile([128, 4, H2], BF16)
            W3 = wts.tile([128, 4, H2], BF16)
            Wd = wts.tile([128, 4, H], F32)
            Wo = wts.tile([128, 2, L], F32)
            nc.sync.dma_start(W1, W1d[:, :, :])
            nc.sync.dma_start(W2, W2d[:, :, :])
            nc.sync.dma_start(W3, W3d[:, :, :])
            nc.sync.dma_start(Wd, Wdd[:, :, :])
            nc.sync.dma_start(Wo, Wod[:, :, :])
            breps = wts.tile([128, 3, H2], F32)
            nc.sync.dma_start(breps, brepd[:, :, :])
            bdrow = wts.tile([1, H], F32)
            borow = wts.tile([1, L], F32)
            nc.sync.dma_start(bdrow, bdd[:, :])
            nc.sync.dma_start(borow, bod[:, :])

            # per-graph node-major features (fp16 + f32 copy)
            x0h = [st.tile([128, 8, H], FEAT_DT, name=f"x0h_{g}") for g in range(GPC)]
            x0 = [st.tile([128, 8, H], F32, name=f"x0_{g}") for g in range(GPC)]
            for g in range(GPC):
                nc.sync.dma_start(
                    x0h[g], feats[g].rearrange("(j p) f -> p j f", p=128))
                nc.vector.tensor_copy(
                    x0[g].rearrange("p j f -> p (j f)"),
                    x0h[g].rearrange("p j f -> p (j f)"))

            # ---------------- cdist: nd = -(d2) ----------------
            big = tc.tile_pool(name="big", bufs=1)
            bigp = big.__enter__()
            nd = [bigp.tile([128, 8, N], F32, name=f"nd{g}") for g in range(GPC)]
            n2pp = st.tile([128, GPC, 8], F32)
            cd = tc.tile_pool(name="cdtmp", bufs=1)
            cdp = cd.__enter__()
            n2rep = [cdp.tile([128, N], F32, name=f"n2rep{g}") for g in range(GPC)]
            with (
                tc.tile_pool(name="cwork", bufs=2) as cw,
                tc.tile_pool(name="cpsum", bufs=2, space=bass.MemorySpace.PSUM) as cps,
            ):
                xT = [cdp.tile([128, 2, N], FEAT_DT, name=f"xT_{g}") for g in range(GPC)]
                for g in range(GPC):
                    for k in range(2):
                        nc.sync.dma_start(
                            xT[g][:, k, :],
                            feats[g].rearrange("t (k p) -> p k t", p=128)[:, k, :])
                for g in range(GPC):
                    for j in range(8):
                        dummy = cw.tile([128, H], F32, tag="dummy")
                        nc.vector.scalar_tensor_tensor(
                            dummy, x0[g][:, j, :], 1.0, x0[g][:, j, :],
                            op0=AOP.mult, op1=AOP.mult,
                            accum_out=n2pp[:, g, j:j+1])
                    # bounce n2 to row form, then replicate across partitions
                    nc.sync.dma_start(
                        rowscr[g][0:N].rearrange("(j p) -> p j", p=128),
                        n2pp[:, g, :])
                    n2row = cw.tile([1, N], F32, tag="n2row")
                    nc.sync.dma_start(n2row, rowscr[g][None, 0:N])
                    n2ps = cps.tile([128, N], F32, tag="n2ps")
                    nc.tensor.matmul(n2ps[:, 0:512], onesRow, n2row[:, 0:512],
                                     start=True, stop=True)
                    nc.tensor.matmul(n2ps[:, 512:N], onesRow, n2row[:, 512:N],
                                     start=True, stop=True)
                    nc.vector.tensor_copy(n2rep[g], n2ps)
                for g in range(GPC):
                    for tj in range(8):
                        for cc in range(2):
                            csl = slice(cc * 512, (cc + 1) * 512)
                            mps = cps.tile([128, 512], F32, tag="mps")
                            for k in range(2):
                                nc.tensor.matmul(
                                    mps, xT[g][:, k, tj * 128:(tj + 1) * 128],
                                    xT[g][:, k, csl],
                                    start=(k == 0), stop=(k == 1))
                            t1 = cw.tile([128, 512], F32, tag="t1")
                            # t1 = 2*dot - n2col
                            nc.vector.scalar_tensor_tensor(
                                t1, mps, 2.0, n2rep[g][:, csl],
                                op0=AOP.mult, op1=AOP.subtract)
                            # nd = t1 - n2row(per-partition)
                            nc.vector.tensor_scalar(
                                nd[g][:, tj, csl], t1, n2pp[:, g, tj:tj+1], None,
                                op0=AOP.subtract)

            cd.__exit__(None, None, None)
            if _SECTIONS == "cdist":
                dummy_out = st.tile([GPC, L], F32)
                nc.vector.tensor_copy(dummy_out, n2pp[0:GPC, 0, :])
                nc.sync.dma_start(outd[:, :], dummy_out)
            # ---------------- Prim (fused both graphs) ----------------
            maxd = st.tile([128, GPC, 8], F32)
            treeNEG = st.tile([128, GPC, 8], F32)
            parent = st.tile([128, GPC, 8], F32)
            wsq_all = st.tile([128, GPC, 8], F32)
            if _SECTIONS != "cdist":
                nc.vector.memset(treeNEG, 0.0)
                nc.vector.memset(parent, 0.0)
                for g in range(GPC):
                    nc.vector.tensor_copy(maxd[:, g, :], nd[g][:, :, 0])
                    nc.vector.memset(treeNEG[0:1, g, 0:1], NEG)
            iotaN_b = iotaN[:, None, :].broadcast_to([128, GPC, 8])
            vload_regs = [nc.vector.alloc_register(f"vload{g}") for g in range(GPC)]
            vload_svs = [
                nc.vector.snap(vload_regs[g], True, min_val=0, max_val=N - 1)
                for g in range(GPC)
            ]

            with (
                tc.tile_pool(name="pwork", bufs=2) as wk,
                tc.tile_pool(name="ppsum", bufs=1, space=bass.MemorySpace.PSUM) as pps,
            ):
                NPAD = 32 * (GPC - 1) + 1  # graph g scalars at partition 32*g
                d = wk.tile([128, GPC, 8], F32, tag="d")
                rp1 = st.tile([128, NPAD], F32)
                rp2 = st.tile([128, NPAD], F32)
                nc.vector.memset(rp1, NEG)
                nc.vector.memset(rp2, 0.0)
                tp1 = pps.tile([NPAD, 128], F32, tag="tp1", name="tp1")
                tp2 = pps.tile([NPAD, 128], F32, tag="tp2", name="tp2")
                sc = wk.tile([NPAD, 2], F32, tag="sc")
                scI = wk.tile([NPAD, 1], I32, tag="scI")
                bc = pps.tile([128, 4], F32, tag="bc")
                newd = wk.tile([128, GPC, 8], F32, tag="newd")
                newdM = wk.tile([128, GPC, 8], F32, tag="newdM")
                eqvU = wk.tile([128, GPC, 8], U32, tag="eqvU")
                vsel = wk.tile([128, GPC, 8], F32, tag="vsel")
                updU = wk.tile([128, GPC, 8], U32, tag="updU")

                def prim_iter():
                    nc.vector.tensor_tensor(d, maxd, treeNEG, op=AOP.add)
                    for g in range(GPC):
                        nc.vector.tensor_reduce(
                            rp1[:, 32*g:32*g+1], d[:, g, :], AX.X, AOP.max)
                    nc.tensor.transpose(tp1, rp1, ident)
                    nc.vector.tensor_reduce(sc[:, 0:1], tp1, AX.X, AOP.max)
                    for g in range(GPC):
                        nc.tensor.matmul(bc[:, g:g+1],
                                         onesRowP[32*g:32*g+1, :],
                                         sc[32*g:32*g+1, 0:1],
                                         start=True, stop=True)
                    nc.vector.tensor_tensor(
                        eqvU, d, bc[:, 0:GPC][:, :, None].broadcast_to([128, GPC, 8]),
                        op=AOP.is_equal)
                    nc.vector.tensor_tensor(vsel, eqvU, iotaN_b, op=AOP.mult)
                    for g in range(GPC):
                        nc.vector.tensor_reduce(
                            rp2[:, 32*g:32*g+1], vsel[:, g, :], AX.X, AOP.max)
                    nc.tensor.transpose(tp2, rp2, ident)
                    nc.vector.tensor_reduce(sc[:, 1:2], tp2, AX.X, AOP.max)
                    nc.vector.tensor_copy(scI, sc[:, 1:2])
                    for g in range(GPC):
                        nc.vector.reg_load(vload_regs[g], scI[32*g:32*g+1, 0:1])
                        nc.vector.tensor_copy(
                            newd[:, g, :][:, :, None],
                            nd[g][:, :, ds(vload_svs[g], 1)])
                        nc.tensor.matmul(bc[:, 2+g:3+g],
                                         onesRowP[32*g:32*g+1, :],
                                         sc[32*g:32*g+1, 1:2],
                                         start=True, stop=True)
                    eqv2U = wk.tile([128, GPC, 8], U32, tag="eqv2U")
                    nc.vector.tensor_tensor(
                        eqv2U, iotaN_b,
                        bc[:, 2:4][:, :, None].broadcast_to([128, GPC, 8]),
                        op=AOP.is_equal)
                    nc.vector.scalar_tensor_tensor(treeNEG, eqv2U, NEG, treeNEG,
                                                   op0=AOP.mult, op1=AOP.add)
                    nc.vector.tensor_tensor(newdM, newd, treeNEG, op=AOP.add)
                    nc.vector.tensor_tensor(updU, newdM, maxd, op=AOP.is_gt)
                    nc.vector.copy_predicated(
                        parent, updU,
                        bc[:, 2:4][:, :, None].broadcast_to([128, GPC, 8]))
                    nc.vector.tensor_tensor(maxd, maxd, newd, op=AOP.max)

                if _SECTIONS != "cdist":
                    n_outer, rem = divmod(n_prim, UNROLL)
                    if n_outer > 0:
                        with tc.For_i(0, n_outer, 1, hint_engines=(DVE,)) as _oi:
                            for _ in range(UNROLL):
                                prim_iter()
                    for _ in range(rem):
                        prim_iter()

            # wsq_all[v] = -w[v]^2 = sum_t (t == parent[v]) * nd[v, t],
            # reconstructed post-loop (removes 2 DVE ops from every Prim iter).
            if _SECTIONS != "cdist":
                with tc.tile_pool(name="wrec", bufs=1) as wr:
                    for g in range(GPC):
                        pmask = wr.tile([128, 8, N], F32, tag="pmask")
                        for uj in range(8):
                            nc.vector.tensor_scalar(
                                pmask[:, uj, :], iotaR,
                                parent[:, g, uj:uj+1], None, op0=AOP.is_equal)
                        nc.vector.tensor_tensor(pmask, pmask, nd[g], op=AOP.mult)
                        nc.vector.tensor_reduce(
                            wsq_all[:, g, :], pmask, AX.X, AOP.add)
                    # node 0 has no parent edge: zero its slot
                    for g in range(GPC):
                        nc.vector.memset(wsq_all[0:1, g, 0:1], 0.0)
            if _SECTIONS == "prim":
                dummy_out = st.tile([GPC, L], F32)
                nc.vector.tensor_copy(dummy_out, wsq_all[0:GPC, 0, :])
                nc.sync.dma_start(outd[:, :], dummy_out)

            big.__exit__(None, None, None)
            # ---------------- post-Prim + layers per graph ----------------
            for g in range(GPC if _SECTIONS == "full" else 0):
                with (
                    tc.tile_pool(name=f"lw{g}", bufs=1) as lw,
                    tc.tile_pool(name=f"lp{g}", bufs=1,
                                 space=bass.MemorySpace.PSUM) as lp,
                ):
                    # w = sqrt(max(-wneg_clamped, 0)); wneg<=0 holds -w^2
                    wsq = lw.tile([128, 8], F32, tag="wsq")
                    wv = lw.tile([128, 8], F32, tag="wv")
                    nc.vector.tensor_scalar_min(wsq, wsq_all[:, g, :], 0.0)
                    nc.scalar.activation(wv, wsq, ACTF.Sqrt, scale=-1.0)

                    # one-hot matrices
                    PARm = lw.tile([128, 8, N], BF16, tag="PARm")
                    CHm = lw.tile([128, 8, N], BF16, tag="CHm")
                    for uj in range(8):
                        nc.vector.tensor_scalar(
                            PARm[:, uj, :], iotaR,
                            parent[:, g, uj:uj+1], None, op0=AOP.is_equal)
                    rowpool_cm = tc.tile_pool(name=f"rows{g}", bufs=1)
                    rw = rowpool_cm.__enter__()
                    rowps_cm = tc.tile_pool(name=f"rowps{g}", bufs=1,
                                            space=bass.MemorySpace.PSUM)
                    rps = rowps_cm.__enter__()
                    # parent row replicated
                    nc.sync.dma_start(
                        rowscr[g][0:N].rearrange("(j p) -> p j", p=128),
                        parent[:, g, :])
                    prow = rw.tile([1, N], F32, tag="prow")
                    nc.sync.dma_start(prow, rowscr[g][None, 0:N])
                    prep_ps = rps.tile([128, N], F32, tag="prep_ps")
                    nc.tensor.matmul(prep_ps[:, 0:512], onesRow, prow[:, 0:512],
                                     start=True, stop=True)
                    nc.tensor.matmul(prep_ps[:, 512:N], onesRow, prow[:, 512:N],
                                     start=True, stop=True)
                    prep = rw.tile([128, N], F32, tag="prep")
                    nc.vector.tensor_copy(prep, prep_ps)
                    for uj in range(8):
                        nc.vector.tensor_scalar(
                            CHm[:, uj, :], prep, iotaN[:, uj:uj+1], None,
                            op0=AOP.is_equal)

                    # degree via scatter matmul: contrib[t] = sum_u w[u] PAR[u,t]
                    wb = lw.tile([128, 8], BF16, tag="wb")
                    nc.vector.tensor_copy(wb, wv)
                    drow_ps = rps.tile([1, N], F32, tag="drow_ps")
                    for cc in range(2):
                        csl = slice(cc * 512, (cc + 1) * 512)
                        for uj in range(8):
                            nc.tensor.matmul(
                                drow_ps[:, csl], wb[:, uj:uj+1],
                                PARm[:, uj, csl],
                                start=(uj == 0),
                                stop=(uj == 7))
                    # w row
                    nc.sync.dma_start(
                        rowscr[g][0:N].rearrange("(j p) -> p j", p=128), wv)
                    wrow = rw.tile([1, N], F32, tag="wrow")
                    nc.sync.dma_start(wrow, rowscr[g][None, 0:N])
                    # deg = 1 + wrow + contrib ; rows: coefficients
                    crow = rw.tile([1, 5, N], F32, tag="crow")
                    deg = rw.tile([1, N], F32, tag="deg")
                    nc.vector.tensor_tensor(deg, drow_ps, wrow, op=AOP.add)
                    nc.vector.tensor_scalar_add(deg, deg, 1.0)
                    sq = rw.tile([1, N], F32, tag="sq")
                    nc.scalar.activation(sq, deg, ACTF.Sqrt)
                    dinv = crow[:, 0, :]
                    nc.vector.reciprocal(dinv, sq)
                    # c1 = alpha + (1-alpha) dinv^2 ; c2=(1-a) w dinv; c3=(1-a)dinv
                    # ycoef = w*dinv
                    nc.vector.scalar_tensor_tensor(
                        crow[:, 1, :], dinv, 1.0 - ALPHA, dinv,
                        op0=AOP.mult, op1=AOP.mult)
                    nc.vector.tensor_scalar_add(crow[:, 1, :], crow[:, 1, :], ALPHA)
                    nc.vector.tensor_tensor(crow[:, 4, :], wrow, dinv, op=AOP.mult)
                    nc.vector.tensor_scalar(crow[:, 2, :], crow[:, 4, :],
                                            1.0 - ALPHA, None, op0=AOP.mult)
                    nc.vector.tensor_scalar(crow[:, 3, :], dinv, 1.0 - ALPHA,
                                            None, op0=AOP.mult)
                    # bounce coeff rows to per-partition form [128, 5, 8]
                    nc.sync.dma_start(
                        rowscr[g][None, 0:5 * N],
                        crow.rearrange("a k t -> a (k t)"))
                    cpp = lw.tile([128, 5, 8], F32, tag="cpp")
                    nc.sync.dma_start(
                        cpp, rowscr[g][0:5 * N].rearrange("(k j p) -> p k j", p=128, k=5))
                    rowps_cm.__exit__(None, None, None)
                    rowpool_cm.__exit__(None, None, None)
                    lypool_cm = tc.tile_pool(name=f"ly{g}", bufs=1)
                    ly = lypool_cm.__enter__()
                    dinv_pp = cpp[:, 0, :]
                    c1_pp = cpp[:, 1, :]
                    c2_pp = cpp[:, 2, :]
                    c3_pp = cpp[:, 3, :]
                    yc_pp = cpp[:, 4, :]

                    # ---------------- 3 SSG layers ----------------
                    x_cur = x0[g]
                    for li, (Wt, nk, fin, fout) in enumerate(
                        ((W1, 2, H, H2), (W2, 4, H2, H2), (W3, 4, H2, H2))
                    ):
                        xsb = ly.tile([128, 8, fin], BF16, tag="xsb", name=f"xsb{g}{li}")
                        yvb = ly.tile([128, 8, fin], BF16, tag="yvb", name=f"yvb{g}{li}")
                        ht = ly.tile([128, 8, fin], F32, tag="ht", name=f"ht{g}{li}")
                        for j in range(8):
                            nc.vector.tensor_scalar(
                                xsb[:, j, :], x_cur[:, j, :], dinv_pp[:, j:j+1],
                                None, op0=AOP.mult)
                            nc.vector.tensor_scalar(
                                yvb[:, j, :], x_cur[:, j, :], yc_pp[:, j:j+1],
                                None, op0=AOP.mult)
                        for tj in range(8):
                            gx = lp.tile([128, fin], F32, tag="gx", name=f"gx{g}{li}{tj}")
                            g2 = lp.tile([128, fin], F32, tag="g2", name=f"g2{g}{li}{tj}")
                            tsl = slice(tj * 128, (tj + 1) * 128)
                            for uk in range(8):
                                nc.tensor.matmul(
                                    gx, CHm[:, uk, tsl], xsb[:, uk, :],
                                    start=(uk == 0), stop=(uk == 7))
                            for uk in range(8):
                                nc.tensor.matmul(
                                    g2, PARm[:, uk, tsl], yvb[:, uk, :],
                                    start=(uk == 0), stop=(uk == 7))
                            nc.vector.tensor_scalar(
                                ht[:, tj, :], x_cur[:, tj, :], c1_pp[:, tj:tj+1],
                                None, op0=AOP.mult)
                            nc.vector.scalar_tensor_tensor(
                                ht[:, tj, :], gx, c2_pp[:, tj:tj+1], ht[:, tj, :],
                                op0=AOP.mult, op1=AOP.add)
                            nc.vector.scalar_tensor_tensor(
                                ht[:, tj, :], g2, c3_pp[:, tj:tj+1], ht[:, tj, :],
                                op0=AOP.mult, op1=AOP.add)
                        # transpose ht -> hT [128, fin/128, N] (bf16)
                        hT = ly.tile([128, 4, N], BF16, tag="hT", name=f"hT{g}{li}")
                        for tj in range(8):
                            for fk in range(fin // 128):
                                tps = lp.tile([128, 128], F32, tag="tps")
                                nc.tensor.transpose(
                                    tps, ht[:, tj, fk * 128:(fk + 1) * 128], ident)
                                nc.vector.tensor_copy(
                                    hT[:, fk, tj * 128:(tj + 1) * 128], tps)
                        # x_next = tanh(h @ W + b)
                        x_next = ly.tile([128, 8, fout], F32, tag="xn2" if li % 2 else "xn1",
                                         name=f"xn{g}{li}")
                        for tj in range(8):
                            xps = lp.tile([128, fout], F32, tag="xps")
                            tsl = slice(tj * 128, (tj + 1) * 128)
                            for fk in range(fin // 128):
                                nc.tensor.matmul(
                                    xps, hT[:, fk, tsl], Wt[:, fk, :],
                                    start=(fk == 0), stop=(fk == fin // 128 - 1))
                            nc.vector.tensor_tensor(
                                x_next[:, tj, :], xps,
                                breps[:, li, 0:fout], op=AOP.add)
                            nc.scalar.activation(
                                x_next[:, tj, :], x_next[:, tj, :], ACTF.Tanh)
                        x_cur = x_next

                    # ---------------- pool + head ----------------
                    pool_ps = lp.tile([1, H2], F32, tag="gx", name="pool_ps")
                    for tj in range(8):
                        nc.tensor.matmul(pool_ps, onesCol, x_cur[:, tj, :],
                                         start=(tj == 0), stop=(tj == 7))
                    pooled = ly.tile([1, H2], F32, tag="pooled")
                    nc.vector.tensor_scalar(pooled, pool_ps, 1.0 / N, None,
                                            op0=AOP.mult)
                    pcol = ly.tile([128, 4], F32, tag="pcol")
                    for fk in range(4):
                        tpp = lp.tile([128, 128], F32, tag="tps", name="tpp")
                        nc.tensor.transpose(
                            tpp, pooled[:, fk * 128:(fk + 1) * 128], ident[0:1, :])
                        nc.vector.tensor_copy(pcol[:, fk:fk+1], tpp[:, 0:1])
                    h1ps = lp.tile([1, H], F32, tag="g2", name="h1ps")
                    for fk in range(4):
                        nc.tensor.matmul(h1ps, pcol[:, fk:fk+1], Wd[:, fk, :],
                                         start=(fk == 0), stop=(fk == 3))
                    h1 = ly.tile([1, H], F32, tag="h1")
                    nc.vector.tensor_tensor(h1, h1ps, bdrow, op=AOP.add)
                    nc.scalar.activation(h1, h1, ACTF.Tanh)
                    hcol = ly.tile([128, 2], F32, tag="hcol")
                    for fk in range(2):
                        tph = lp.tile([128, 128], F32, tag="tps", name="tph")
                        nc.tensor.transpose(
                            tph, h1[:, fk * 128:(fk + 1) * 128], ident[0:1, :])
                        nc.vector.tensor_copy(hcol[:, fk:fk+1], tph[:, 0:1])
                    ops = lp.tile([1, L], F32, tag="xps", name="ops")
                    for fk in range(2):
                        nc.tensor.matmul(ops, hcol[:, fk:fk+1], Wo[:, fk, :],
                                         start=(fk == 0), stop=(fk == 1))
                    fout_t = ly.tile([1, L], F32, tag="fout_t")
                    nc.vector.tensor_tensor(fout_t, ops, borow, op=AOP.add)
                    nc.sync.dma_start(outd[g][None, :], fout_t)
                    lypool_cm.__exit__(None, None, None)

    _fix_sync_waits(nc)
    return nc


# ---------------- host-side execution ----------------

_WKEYS = ("W1", "b1", "W2", "b2", "W3", "b3", "Wd", "bd", "Wo", "bo")
_STATE = {}
_CPU_CAST = []


def _cast_feats(features):
    """f32 -> FEAT_NP via XLA's CPU backend (~5ms vs ~44ms ml_dtypes astype)."""
    x = np.asarray(features, dtype=np.float32)
    if not _CPU_CAST:
        try:
            import jax.numpy as jnp
            cpu = jax.devices("cpu")[0]
            fn = jax.jit(lambda a: a.astype(FEAT_NP), device=cpu)
            probe = np.asarray(fn(np.ones((4,), np.float32)))
            assert probe.dtype == FEAT_NP
            _CPU_CAST.append(fn)
        except Exception:
            _CPU_CAST.append(None)
    if _CPU_CAST[0] is not None:
        try:
            return np.ascontiguousarray(np.asarray(_CPU_CAST[0](x)))
        except Exception:
            pass
    return np.ascontiguousarray(x.astype(FEAT_NP))


_LAST_IDS = {}


def _weights_key(weights):
    ids = tuple(id(weights[k]) for k in _WKEYS)
    hit = _LAST_IDS.get(ids)
    if hit is not None:
        return hit[0]
    h = hashlib.sha1()
    for k in _WKEYS:
        h.update(np.ascontiguousarray(weights[k]).tobytes())
    key = h.hexdigest()
    # keep strong references so these ids cannot be recycled while cached
    _LAST_IDS[ids] = (key, [weights[k] for k in _WKEYS])
    return key


def _make_sharded(nc):
    """Persistent jitted executable mirroring bass2jax.run_bass_via_pjrt."""
    install_neuronx_cc_hook()
    partition_name = nc.partition_id_tensor.name if nc.partition_id_tensor else None
    in_names, out_names, out_avals, zero_shapes = [], [], [], []
    for alloc in nc.m.functions[0].allocations:
        if not isinstance(alloc, mybir.MemoryLocationSet):
            continue
        name = alloc.memorylocations[0].name
        if alloc.kind == "ExternalInput":
            if name != partition_name:
                in_names.append(name)
        elif alloc.kind == "ExternalOutput":
            out_names.append(name)
            shape = tuple(alloc.tensor_shape)
            dtype = mybir.dt.np(alloc.dtype)
            out_avals.append(jax.core.ShapedArray(shape, dtype))
            zero_shapes.append((shape, dtype))
    n_params = len(in_names)
    n_outs = len(out_avals)
    in_names = in_names + out_names
    if partition_name is not None:
        in_names.append(partition_name)
    donate = tuple(range(n_params, n_params + n_outs))

    def _body(*args):
        operands = list(args)
        if partition_name is not None:
            operands.append(partition_id_tensor())
        outs = _bass_exec_p.bind(
            *operands,
            out_avals=tuple(out_avals),
            in_names=tuple(in_names),
            out_names=tuple(out_names),
            lowering_input_output_aliases=(),
            sim_require_finite=True,
            sim_require_nnan=True,
            nc=nc,
        )
        return tuple(outs)

    devices = jax.devices()[:NCORES]
    mesh = Mesh(np.asarray(devices), ("core",))
    in_specs = (PartitionSpec("core"),) * (n_params + n_outs)
    out_specs = (PartitionSpec("core"),) * len(out_names)
    sharded = jax.jit(
        shard_map(_body, mesh=mesh, in_specs=in_specs, out_specs=out_specs,
                  check_rep=False),
        donate_argnums=donate, keep_unused=True,
    )
    # AOT-compile with concrete avals: warm calls then skip pjit's Python
    # dispatch path (which misses its C fast path on every call here).
    try:
        feat_sds = jax.ShapeDtypeStruct((B, N, H), FEAT_NP)
        zero_sds = [jax.ShapeDtypeStruct((NCORES * s[0], *s[1:]), dt)
                    for s, dt in zero_shapes]
        fn = sharded.lower(feat_sds, *zero_sds).compile()
    except Exception:
        fn = sharded
    return {"fn": fn, "zero_shapes": zero_shapes, "n_outs": n_outs}


def _run_sharded(st, feats16):
    zeros = [np.zeros((NCORES * s[0], *s[1:]), dt) for s, dt in st["zero_shapes"]]
    out_arrs = st["fn"](feats16, *zeros)
    return np.asarray(out_arrs[0])  # global [B, L]


def kernel(features, W1, b1, W2, b2, W3, b3, Wd, bd, Wo, bo, _n_prim=N_PRIM,
           _trace=False):
    weights = {"W1": W1, "b1": b1, "W2": W2, "b2": b2, "W3": W3, "b3": b3,
               "Wd": Wd, "bd": bd, "Wo": Wo, "bo": bo}
    feats16 = _cast_feats(features)
    key = (_weights_key(weights), _n_prim)
    st = _STATE.get(key)
    if st is None or _trace:
        nc = _build(weights, _n_prim)
        in_maps = [{"feats": feats16[c * GPC:(c + 1) * GPC]} for c in range(NCORES)]
        res = run_bass_kernel_spmd(nc, in_maps, list(range(NCORES)), trace=_trace)
        out = np.concatenate([res.results[c]["out"] for c in range(NCORES)], axis=0)
        if _trace:
            kernel._last_exec_time_ns = res.exec_time_ns
            return out
        st = _make_sharded(nc)
        _STATE[key] = st
        try:
            _run_sharded(st, feats16)  # warm the persistent executable
        except Exception:
            _STATE.pop(key, None)
        return out
    return _run_sharded(st, feats16)
